# revision 53
# baseline (speedup 1.0000x reference)
"""NeuralOpinionDynamics Trainium2 kernel (8-core SPMD, row-sharded).

out = omega*relu(A_norm @ Z W_D^T) + (1-omega)*softmax(sigmoid(s_i - s_j)) @ Z W_C^T
      + delta*(tanh(Z W1^T + b1) W2^T + b2),   s = Z @ w_V

Algebraic restructurings vs the direct formulation:

1. Convection/attention is separable: with t = s/S in [-1,1], B_ij =
   exp(sigmoid(s_i - s_j)) is approximated by a degree-14 polynomial in
   (s_i - s_j) expanded binomially, B_ij ~= sum_{l,p} t_i^l M[l,p] t_j^p.
   Then numer_i = sum_j B_ij [zwc_j|t_j] = Phi_i . (M @ H) with moments
   H[p,:] = sum_j t_j^p [zwc_j | t_j]. The whole [N,N] attention collapses
   into 64 accumulating [128,16]x[128,129] moment matmuls + 8 small ones.
   The softmax denominator needs plain moments m_p = sum_j t_j^p; column
   128 of H provides m_{p+1} and m_0 = N is a compile-time constant, so
   G[:,128] = M[:,1:] @ H[:,128] + N*M[:,0] via two extra tiny matmuls.

2. Diffusion uses A_norm = dinv_i * adj_ij * dinv_j with adj binary:
   adj ships as fp8 (0/1 exact, 1 byte -> 4x less HBM than fp32 A_norm)
   and multiplies a bf16 stationary y = (dinv .* Z) W_D'^T on the PE
   (mixed bf16 x fp8 matmul verified exact on HW). The dinv_j row scale
   is folded into a second host-scaled copy of Z^T (zts), so the per-tile
   PSUM drain is a single [128,257] copy of [y | zwc | t].

Schedule: small inputs ride in two packed DMAs; the pre-loop (reaction
MLP, own-row power basis) fills the adj/zt DMA startup; the j-stream
interleaves zt/zts chunks with 1 MiB adj slabs and runs zw + dif matmuls
per tile, with half-stream psi power batches (DVE) and incremental
moment matmuls; the finish phase combines dif/con/rea per i-tile into a
staging tile written out in one DMA. PSUM drains alternate DVE/ACT.
GpSimd is avoided entirely (Q7 tensor ops are ~usec-each on HW).

Sharding: core c owns output rows [1024c, 1024(c+1)); adj symmetric, so
the column slice adj[:, rows_c] (host-swizzled to [128, 64*1024] j-tile
layout) doubles as the row slice. Z^T and small weights replicated; no
collectives.
"""

import sys

sys.path.insert(0, "/opt/trn_rl_repo")

from math import comb

import numpy as np

import concourse.bass as bass
import concourse.mybir as mybir
from concourse import bacc
from concourse.bass_utils import run_bass_kernel_spmd
from concourse.masks import make_identity
from concourse.tile import TileContext

N = 8192
D = 128
NCORES = 8
M = N // NCORES            # rows per core = 1024
JT = N // 128              # j-tiles = 64
IT = M // 128              # i-tiles per core = 8
SLAB = 8                   # j-tiles per adj DMA slab (1 MiB)
NSLAB = JT // SLAB
DEG = 14                   # attention polynomial degree
PP = 16                    # padded power count (DEG+1 -> 16)
YW = 257                   # [y | zwc | t] block width per j-tile

# packed bf16 small-input layout (columns)
W_RHSW = 0                 # [D, 258]
W_W1T = 258                # [D, 128]
W_W2T = 386                # [D, 128]
W_MT = 514                 # [16, 16]
W_MTSH = 530               # [16, 16]
W_ZI = 546                 # [D, 1024]
W_COLS = 1570
# packed fp32 small-input layout (columns)
F_B1 = 0                   # [D, 1]
F_M0 = 1                   # [1, 16] (row 0)
F_B2C = 17                 # [D, 1]
F_DISCB = 18               # [128, 1024] dinv_i broadcast for the core's rows
F_COLS = 1042

FP32 = mybir.dt.float32
BF16 = mybir.dt.bfloat16
FP8 = mybir.dt.float8e4
AF = mybir.ActivationFunctionType
ALU = mybir.AluOpType

LAST_RESULTS = None
LAST_IN_MAPS = None


def build_program(reps=1, hwloop=False, psi_batches=2, zt_in_loop=True,
                  pair_drain=False, slab=SLAB, drain_mode="alt",
                  no_dif=False, no_adj=False):
    nslab = JT // slab
    nc = bacc.Bacc("TRN2", target_bir_lowering=False, debug=False)

    adj_d = nc.dram_tensor("adj", [128, JT * M], FP8, kind="ExternalInput")
    zt_d = nc.dram_tensor("zt", [D, N], BF16, kind="ExternalInput")
    zts_d = nc.dram_tensor("zts", [D, N], BF16, kind="ExternalInput")
    wpack_d = nc.dram_tensor("wpack", [128, W_COLS], BF16, kind="ExternalInput")
    fpack_d = nc.dram_tensor("fpack", [128, F_COLS], FP32, kind="ExternalInput")
    # out^T: the host transposes each core's [D, M] slab back to [M, D]
    out_d = nc.dram_tensor("out", [D, M], FP32, kind="ExternalOutput")

    with TileContext(nc) as tc:
        with (
            tc.tile_pool(name="persist", bufs=1) as pp,
            tc.tile_pool(name="adjp", bufs=3) as ap_pool,
            tc.tile_pool(name="small", bufs=2) as smp,
            tc.tile_pool(name="pzw", bufs=2 if pair_drain else 3,
                         space="PSUM") as pzw,
            tc.tile_pool(name="pdif", bufs=1, space="PSUM") as pdif,
            tc.tile_pool(name="ph", bufs=1, space="PSUM") as ph,
            tc.tile_pool(name="pz", bufs=1 if pair_drain else 2,
                         space="PSUM") as pz,
        ):
            # ---- persistent SBUF ----
            zt = pp.tile([D, N], BF16)
            zts = pp.tile([D, N], BF16)
            wpack = pp.tile([128, W_COLS], BF16)
            fpack = pp.tile([128, F_COLS], FP32)
            onesc = pp.tile([1, 1], FP32)
            ywct = pp.tile([128, JT * YW], BF16)
            t_all = pp.tile([128, JT], FP32)
            t_core = pp.tile([128, IT], FP32)
            psi_f = pp.tile([128, JT * PP], FP32)
            psi_b = pp.tile([128, JT * PP], BF16)
            psi_core = pp.tile([128, IT * PP], FP32)
            reaT = pp.tile([D, M], FP32)
            ht = pp.tile([D, M], BF16)
            pht_all = pp.tile([PP, IT * 128], BF16)
            out_sb = pp.tile([D, M], FP32)
            ones16 = pp.tile([PP, 128], BF16)
            ident = pp.tile([128, 128], FP32)

            rhsw = wpack[:, W_RHSW : W_RHSW + 258]
            w1t = wpack[:, W_W1T : W_W1T + 128]
            w2t = wpack[:, W_W2T : W_W2T + 128]
            mt = wpack[0:PP, W_MT : W_MT + PP]
            mtsh = wpack[0:PP, W_MTSH : W_MTSH + PP]
            zi = wpack[:, W_ZI : W_ZI + M]
            b1 = fpack[:, F_B1 : F_B1 + 1]
            m0row = fpack[0:1, F_M0 : F_M0 + PP]
            b2c = fpack[:, F_B2C : F_B2C + 1]
            discb = fpack[:, F_DISCB : F_DISCB + M]

            nc.scalar.dma_start(wpack[:], wpack_d[:])
            nc.scalar.dma_start(fpack[:], fpack_d[:])
            if not zt_in_loop:
                nc.sync.dma_start(zt[:], zt_d[:])
                nc.sync.dma_start(zts[:], zts_d[:])

            make_identity(nc, ident[:])
            nc.vector.memset(onesc[:], 1.0)
            nc.vector.memset(ones16[:], 1.0)
            # power-basis pads: col 0 = 1.0, cols DEG+1..15 = 0
            nc.vector.memset(psi_f[:], 0.0)
            psi_v = psi_f[:].rearrange("p (t c) -> p t c", c=PP)
            nc.vector.memset(psi_v[:, :, 0], 1.0)
            nc.vector.memset(psi_core[:], 0.0)
            psc_v = psi_core[:].rearrange("p (t c) -> p t c", c=PP)
            nc.vector.memset(psc_v[:, :, 0], 1.0)

            ywct_v = ywct[:].rearrange("p (t c) -> p t c", c=YW)
            adj_fix = None
            if no_adj:
                adj_fix = pp.tile([128, slab * M], FP8)
                nc.vector.memset(adj_fix[:], 1.0)

            # ---- PSUM accumulators (allocated once, cleared by start=True) ----
            ps_dif = [
                pdif.tile([128, 512], FP32, tag=f"dif{h}", name=f"ps_dif{h}")
                for h in range(2)
            ]
            ps_h = ph.tile([PP, 129], FP32)

            import contextlib
            rep_ctx = tc.For_i(0, reps, 1) if hwloop and reps > 1 else None
            for _rep in range(1 if hwloop else reps):
              with (rep_ctx if rep_ctx is not None else contextlib.nullcontext()):
                # ---- pre-loop: own-row t basis + reaction MLP (zi only) ----
                for it in range(IT):
                    tps = pz.tile([128, 2], FP32, tag="mix", name="tps")
                    nc.tensor.matmul(
                        tps[:], zi[:, it * 128 : (it + 1) * 128], rhsw[:, 256:258],
                        start=True, stop=True, skip_group_check=True,
                    )
                    nc.vector.tensor_copy(t_core[:, it : it + 1], tps[:, 0:1])
                nc.vector.tensor_copy(psc_v[:, :, 1], t_core[:])
                for l in range(2, DEG + 1):
                    nc.vector.tensor_mul(
                        psc_v[:, :, l], psc_v[:, :, l - 1], t_core[:]
                    )
                for it in range(IT):
                    phps = pz.tile([PP, 128], FP32, tag="mix", name="phps")
                    nc.tensor.transpose(
                        phps[:], psi_core[:, it * PP : (it + 1) * PP], ident[:]
                    )
                    nc.vector.tensor_copy(
                        pht_all[:, it * 128 : (it + 1) * 128], phps[:]
                    )

                for hh in range(2):
                    t1 = pz.tile([128, 512], FP32, tag="mix", name="t1")
                    nc.tensor.matmul(
                        t1[:], w1t[:], zi[:, hh * 512 : (hh + 1) * 512],
                        start=True, stop=True, skip_group_check=True,
                    )
                    nc.scalar.activation(
                        ht[:, hh * 512 : (hh + 1) * 512], t1[:], AF.Tanh,
                        bias=b1[:], scale=1.0,
                    )
                for hh in range(2):
                    # rea^T = W2' @ tanh(...), b2 folded into the drain
                    rea_ps = pz.tile([128, 512], FP32, tag="mix", name="rea_ps")
                    nc.tensor.matmul(
                        rea_ps[:], w2t[:], ht[:, hh * 512 : (hh + 1) * 512],
                        start=True, stop=True, skip_group_check=True,
                    )
                    nc.vector.tensor_scalar(
                        reaT[:, hh * 512 : (hh + 1) * 512], rea_ps[:],
                        b2c[:], None, op0=ALU.add,
                    )

                # ---- j-tile stream: zw matmuls, merged drain, dif + moments ----
                # dif matmuls run at lag-1 in program order so the PE never
                # waits on its own tile's PSUM drain
                def dif_mms(jt, adjs, q, no_dif=no_dif):
                    if no_dif and jt > 0:
                        return
                    for h in range(2):
                        nc.tensor.matmul(
                            ps_dif[h][:],
                            ywct[:, jt * YW : jt * YW + 128],
                            adjs[:, q * M + h * 512 : q * M + (h + 1) * 512],
                            start=(jt == 0),
                            stop=(jt == JT - 1 or no_dif),
                            skip_group_check=True,
                        )

                prev = None
                for g in range(nslab):
                    # zt/zts chunks interleaved with adj slabs, 2 slabs per chunk
                    if zt_in_loop and g % 2 == 0:
                        csl = slice(g * slab * 128, (g + 2) * slab * 128)
                        nc.sync.dma_start(zt[:, csl], zt_d[:, csl])
                        nc.sync.dma_start(zts[:, csl], zts_d[:, csl])
                    if no_adj:
                        adjs = adj_fix
                    else:
                        adjs = ap_pool.tile([128, slab * M], FP8, tag="adj")
                        nc.sync.dma_start(
                            adjs[:], adj_d[:, g * slab * M : (g + 1) * slab * M]
                        )
                    if pair_drain:
                      for q in range(0, slab, 2):
                        # j-tile pair in one 2-bank PSUM tile, drained by a
                        # single strided copy (alternating DVE/ACT per pair)
                        zw2 = pzw.tile([128, 1024], FP32, tag="zw")
                        for u in range(2):
                            jt = g * slab + q + u
                            off = u * 512
                            nc.tensor.matmul(
                                zw2[:, off : off + 128],
                                zts[:, jt * 128 : (jt + 1) * 128],
                                rhsw[:, 0:128],
                                start=True, stop=False, skip_group_check=True,
                            )
                            nc.tensor.matmul(
                                zw2[:, off + 128 : off + 257],
                                zt[:, jt * 128 : (jt + 1) * 128],
                                rhsw[:, 128:257],
                                start=False, stop=True, skip_group_check=True,
                            )
                        jt0 = g * slab + q
                        zw2_v = zw2[:].rearrange("p (u c) -> p u c", c=512)
                        dst = ywct[:, jt0 * YW : (jt0 + 2) * YW].rearrange(
                            "p (u c) -> p u c", c=YW
                        )
                        if (jt0 // 2) % 2 == 0:
                            nc.vector.tensor_copy(dst, zw2_v[:, :, 0:YW])
                        else:
                            nc.scalar.copy(dst, zw2_v[:, :, 0:YW])
                        for u in range(2):
                            jt = jt0 + u
                            if prev is not None:
                                dif_mms(*prev)
                            prev = (jt, adjs, q + u)
                    else:
                      for q in range(slab):
                        jt = g * slab + q
                        zw = pzw.tile([128, YW], FP32, tag="zw")
                        nc.tensor.matmul(
                            zw[:, 0:128], zts[:, jt * 128 : (jt + 1) * 128],
                            rhsw[:, 0:128],
                            start=True, stop=False, skip_group_check=True,
                        )
                        nc.tensor.matmul(
                            zw[:, 128:257], zt[:, jt * 128 : (jt + 1) * 128],
                            rhsw[:, 128:257],
                            start=False, stop=True, skip_group_check=True,
                        )
                        use_dve = (
                            jt % 2 == 0 if drain_mode == "alt"
                            else drain_mode == "dve"
                        )
                        if use_dve:
                            nc.vector.tensor_copy(
                                ywct[:, jt * YW : (jt + 1) * YW], zw[:]
                            )
                        else:
                            nc.scalar.copy(
                                ywct[:, jt * YW : (jt + 1) * YW], zw[:]
                            )
                        if prev is not None:
                            dif_mms(*prev)
                        prev = (jt, adjs, q)
                    # psi powers (DVE, batched) + moment matmuls, every
                    # NSLAB//psi_batches slabs so H stays incremental
                    per = nslab // psi_batches
                    if (g + 1) % per == 0:
                        bsl = slice((g + 1 - per) * slab, (g + 1) * slab)
                        nc.vector.tensor_copy(t_all[:, bsl], ywct_v[:, bsl, 256])
                        nc.vector.tensor_copy(psi_v[:, bsl, 1], t_all[:, bsl])
                        for l in range(2, DEG + 1):
                            nc.vector.tensor_mul(
                                psi_v[:, bsl, l], psi_v[:, bsl, l - 1],
                                t_all[:, bsl],
                            )
                        nc.vector.tensor_copy(
                            psi_b[:, bsl.start * PP : bsl.stop * PP],
                            psi_f[:, bsl.start * PP : bsl.stop * PP],
                        )
                        for jt in range(bsl.start, bsl.stop):
                            nc.tensor.matmul(
                                ps_h[:],
                                psi_b[:, jt * PP : (jt + 1) * PP],
                                ywct[:, jt * YW + 128 : (jt + 1) * YW],
                                start=(jt == 0), stop=(jt == JT - 1),
                                skip_group_check=True,
                            )

                if prev is not None:
                    dif_mms(*prev)

                # ---- G = M @ H (+ denominator from shifted moments) ----
                hsb = smp.tile([PP, 129], BF16, tag="hsb")
                nc.vector.tensor_copy(hsb[:], ps_h[:])
                gp = pz.tile([PP, 129], FP32, tag="mix", name="gp")
                nc.tensor.matmul(gp[:, 0:128], mt[:], hsb[:, 0:128],
                                 start=True, stop=False, skip_group_check=True)
                nc.tensor.matmul(gp[:, 128:129], mtsh[:], hsb[:, 128:129],
                                 start=False, stop=False, skip_group_check=True)
                nc.tensor.matmul(gp[:, 128:129], m0row[:], onesc[:],
                                 start=False, stop=True, skip_group_check=True)
                gsb = smp.tile([PP, 129], BF16, tag="gsb")
                nc.vector.tensor_copy(gsb[:], gp[:])

                # ---- finish (all in [D, M] orientation, no transposes) ----
                # con^T numerator, denominator broadcast, reciprocal: all
                # independent of the dif stream, so they overlap its tail
                gbc = smp.tile([PP, 128], BF16, tag="gbc")
                nc.vector.tensor_scalar(
                    gbc[:], ones16[:], gp[:, 128:129], None, op0=ALU.mult
                )
                cps, rps = [], []
                cpool = pzw if pair_drain else pz
                ctag = "zw" if pair_drain else "mix"
                for h in range(2):
                    conp = cpool.tile([128, 512], FP32, tag=ctag, name="conp")
                    nc.tensor.matmul(
                        conp[:], gsb[:, 0:128],
                        pht_all[:, h * 512 : (h + 1) * 512],
                        start=True, stop=True, skip_group_check=True,
                    )
                    denp = cpool.tile([128, 512], FP32, tag=ctag, name="denp")
                    nc.tensor.matmul(
                        denp[:], gbc[:], pht_all[:, h * 512 : (h + 1) * 512],
                        start=True, stop=True, skip_group_check=True,
                    )
                    rcph = smp.tile([128, 512], FP32, tag="rcp")
                    nc.vector.reciprocal(rcph[:], denp[:])
                    cps.append(conp)
                    rps.append(rcph)
                # dif-dependent tail: relu(dif^T) . dinv_i + con + rea
                for h in range(2):
                    sl = slice(h * 512, (h + 1) * 512)
                    dd = smp.tile([128, 512], FP32, tag="dd")
                    nc.vector.tensor_tensor(
                        dd[:], ps_dif[h][:], discb[:, sl], op=ALU.mult
                    )
                    o1 = smp.tile([128, 512], FP32, tag="o1")
                    nc.vector.tensor_scalar(o1[:], dd[:], 0.0, None, op0=ALU.max)
                    cc = smp.tile([128, 512], FP32, tag="cc")
                    nc.vector.tensor_tensor(cc[:], cps[h][:], rps[h][:], op=ALU.mult)
                    o2 = smp.tile([128, 512], FP32, tag="o2")
                    nc.vector.tensor_add(o2[:], o1[:], cc[:])
                    nc.vector.tensor_add(out_sb[:, sl], o2[:], reaT[:, sl])
                nc.sync.dma_start(out_d[:], out_sb[:])

    nc.compile()
    return nc


def _sigmoid(x):
    return 1.0 / (1.0 + np.exp(-np.float64(x)))


def prep_inputs(Z, A_norm, W_D, W_C, w_V, W1, b1, W2, b2, omega_logit, delta_logit):
    import ml_dtypes

    bf16 = ml_dtypes.bfloat16
    f8 = mybir.dt.np(FP8)

    Z = np.asarray(Z, dtype=np.float32)
    A_norm = np.asarray(A_norm, dtype=np.float32)
    omega = _sigmoid(omega_logit)
    delta = _sigmoid(delta_logit)

    # attention polynomial: fit exp(sigmoid(x)) on x = s_i - s_j via
    # Chebyshev in u = (t_i - t_j)/2, t = s/S, then binomial expansion
    # into the separable coefficient matrix Mm[l, p].
    s = Z.astype(np.float64) @ np.asarray(w_V, np.float64)
    S = float(np.max(np.abs(s))) * 1.01
    cheb = np.polynomial.chebyshev.Chebyshev.interpolate(
        lambda u: np.exp(_sigmoid(2.0 * S * u)), DEG, domain=[-1, 1]
    )
    p = cheb.convert(kind=np.polynomial.Polynomial).coef
    Mm = np.zeros((PP, PP))
    for k in range(DEG + 1):
        for l in range(k + 1):
            Mm[l, k - l] += p[k] * (0.5 ** k) * comb(k, l) * ((-1.0) ** (k - l))

    # binary adjacency + degree factors
    mask = A_norm != 0
    degc = mask.sum(axis=1)
    dinv = np.where(degc > 0, 1.0 / np.sqrt(np.maximum(degc, 1)), 0.0).astype(
        np.float32
    )
    adj8 = mask.astype(f8)

    zt = np.ascontiguousarray(Z.T).astype(bf16)                    # [D, N]
    zts = np.ascontiguousarray(Z.T * dinv[None, :]).astype(bf16)   # dinv-scaled

    # packed bf16 small inputs (shared part; zi appended per core)
    wpack = np.zeros((128, W_COLS), dtype=bf16)
    wpack[:, W_RHSW : W_RHSW + 128] = (omega * np.asarray(W_D, np.float64)).T
    wpack[:, W_RHSW + 128 : W_RHSW + 256] = (
        (1.0 - omega) * np.asarray(W_C, np.float64)
    ).T
    wpack[:, W_RHSW + 256] = np.asarray(w_V, np.float64) / S
    wpack[:, W_W1T : W_W1T + 128] = np.asarray(W1, np.float64).T
    wpack[:, W_W2T : W_W2T + 128] = (delta * np.asarray(W2, np.float64)).T
    wpack[0:PP, W_MT : W_MT + PP] = Mm.T
    Msh = np.zeros((PP, PP))
    Msh[0 : PP - 1, :] = Mm.T[1:PP, :]
    wpack[0:PP, W_MTSH : W_MTSH + PP] = Msh

    fpack = np.zeros((128, F_COLS), dtype=np.float32)
    fpack[:, F_B1] = np.asarray(b1, np.float32)
    fpack[0, F_M0 : F_M0 + PP] = (Mm[:, 0] * float(N)).astype(np.float32)
    fpack[:, F_B2C] = (delta * np.asarray(b2, np.float64)).astype(np.float32)

    shared = {"zt": zt, "zts": zts}
    in_maps = []
    for c in range(NCORES):
        sl = slice(c * M, (c + 1) * M)
        # column slice, swizzled to j-tile layout [128, JT*M]
        adj_c = adj8[:, sl].reshape(JT, 128, M).transpose(1, 0, 2).reshape(
            128, JT * M
        )
        wp = wpack.copy()
        wp[:, W_ZI : W_ZI + M] = zt[:, sl]
        fp = fpack.copy()
        fp[:, F_DISCB : F_DISCB + M] = dinv[sl][None, :]
        in_maps.append({
            **shared,
            "adj": np.ascontiguousarray(adj_c),
            "wpack": np.ascontiguousarray(wp),
            "fpack": np.ascontiguousarray(fp),
        })
    return in_maps


def kernel(Z, A_norm, W_D, W_C, w_V, W1, b1, W2, b2, omega_logit, delta_logit):
    global LAST_RESULTS, LAST_IN_MAPS
    in_maps = prep_inputs(
        Z, A_norm, W_D, W_C, w_V, W1, b1, W2, b2, omega_logit, delta_logit
    )
    LAST_IN_MAPS = in_maps
    nc = build_program()
    LAST_RESULTS = run_bass_kernel_spmd(nc, in_maps, list(range(NCORES)))
    # device emits out^T [D, M] per core
    return np.concatenate(
        [LAST_RESULTS.results[c]["out"].T for c in range(NCORES)], axis=0
    )


# revision 56
# speedup vs baseline: 2.5552x; 2.5552x over previous
"""NeuralOpinionDynamics Trainium2 kernel (8-core SPMD, row-sharded).

out = omega*relu(A_norm @ Z W_D^T) + (1-omega)*softmax(sigmoid(s_i - s_j)) @ Z W_C^T
      + delta*(tanh(Z W1^T + b1) W2^T + b2),   s = Z @ w_V

Algebraic restructurings vs the direct formulation:

1. Convection/attention is separable: with t = s/S in [-1,1], B_ij =
   exp(sigmoid(s_i - s_j)) is approximated by a degree-14 polynomial in
   (s_i - s_j) expanded binomially, B_ij ~= sum_{l,p} t_i^l M[l,p] t_j^p.
   Then numer_i = sum_j B_ij [zwc_j|t_j] = Phi_i . (M @ H) with moments
   H[p,:] = sum_j t_j^p [zwc_j | t_j]. The whole [N,N] attention collapses
   into 64 accumulating [128,16]x[128,129] moment matmuls + 8 small ones.
   The softmax denominator needs plain moments m_p = sum_j t_j^p; column
   128 of H provides m_{p+1} and m_0 = N is a compile-time constant, so
   G[:,128] = M[:,1:] @ H[:,128] + N*M[:,0] via two extra tiny matmuls.

2. Diffusion uses A_norm = dinv_i * adj_ij * dinv_j with adj binary:
   adj ships as fp8 (0/1 exact, 1 byte -> 4x less HBM than fp32 A_norm)
   and multiplies a bf16 stationary y = (dinv .* Z) W_D'^T on the PE
   (mixed bf16 x fp8 matmul verified exact on HW). The dinv_j row scale
   is folded into a second host-scaled copy of Z^T (zts), so the per-tile
   PSUM drain is a single [128,257] copy of [y | zwc | t].

Schedule: small inputs ride in two packed DMAs; the pre-loop (reaction
MLP, own-row power basis) fills the adj/zt DMA startup; the j-stream
interleaves zt/zts chunks with 1 MiB adj slabs and runs zw + dif matmuls
per tile (dif at lag-1 in program order so the PE never waits on its own
tile's PSUM drain), with half-stream psi power batches (DVE) and
incremental moment matmuls. The finish runs entirely in the transposed
[D, M] orientation — con^T, its denominator broadcast, and rea^T are
bulk N=512 matmuls against the pre-transposed power basis (pht_all), so
no per-i-tile chains or PE transposes remain — and the host transposes
the single [D, M] output back. PSUM drains alternate DVE/ACT. GpSimd is
avoided entirely (Q7 tensor ops are ~usec-each on HW).

Sharding: core c owns output rows [1024c, 1024(c+1)); adj symmetric, so
the column slice adj[:, rows_c] (host-swizzled to [128, 64*1024] j-tile
layout) doubles as the row slice. Z^T and small weights replicated; no
collectives.
"""

import sys

sys.path.insert(0, "/opt/trn_rl_repo")

from math import comb

import numpy as np

import concourse.bass as bass
import concourse.mybir as mybir
from concourse import bacc
from concourse.bass_utils import run_bass_kernel_spmd
from concourse.masks import make_identity
from concourse.tile import TileContext

N = 8192
D = 128
NCORES = 8
M = N // NCORES            # rows per core = 1024
JT = N // 128              # j-tiles = 64
IT = M // 128              # i-tiles per core = 8
SLAB = 8                   # j-tiles per adj DMA slab (1 MiB)
NSLAB = JT // SLAB
DEG = 14                   # attention polynomial degree
PP = 16                    # padded power count (DEG+1 -> 16)
YW = 257                   # [y | zwc | t] block width per j-tile

# packed bf16 small-input layout (columns)
W_RHSW = 0                 # [D, 258]
W_W1T = 258                # [D, 128]
W_W2T = 386                # [D, 128]
W_MT = 514                 # [16, 16]
W_MTSH = 530               # [16, 16]
W_ZI = 546                 # [D, 1024]
W_COLS = 1570
# packed fp32 small-input layout (columns)
F_B1 = 0                   # [D, 1]
F_M0 = 1                   # [1, 16] (row 0)
F_B2C = 17                 # [D, 1]
F_DISCB = 18               # [128, 1024] dinv_i broadcast for the core's rows
F_COLS = 1042

FP32 = mybir.dt.float32
BF16 = mybir.dt.bfloat16
FP8 = mybir.dt.float8e4
AF = mybir.ActivationFunctionType
ALU = mybir.AluOpType

LAST_RESULTS = None
LAST_IN_MAPS = None


def build_program(reps=1, hwloop=False, psi_batches=2, zt_in_loop=True,
                  pair_drain=False, slab=SLAB, drain_mode="alt",
                  adj_bufs=3, no_dif=False, no_adj=False):
    nslab = JT // slab
    nc = bacc.Bacc("TRN2", target_bir_lowering=False, debug=False)

    adj_d = nc.dram_tensor("adj", [128, JT * M], FP8, kind="ExternalInput")
    zt_d = nc.dram_tensor("zt", [D, N], BF16, kind="ExternalInput")
    zts_d = nc.dram_tensor("zts", [D, N], BF16, kind="ExternalInput")
    wpack_d = nc.dram_tensor("wpack", [128, W_COLS], BF16, kind="ExternalInput")
    fpack_d = nc.dram_tensor("fpack", [128, F_COLS], FP32, kind="ExternalInput")
    # out^T: the host transposes each core's [D, M] slab back to [M, D]
    out_d = nc.dram_tensor("out", [D, M], FP32, kind="ExternalOutput")

    with TileContext(nc) as tc:
        with (
            tc.tile_pool(name="persist", bufs=1) as pp,
            tc.tile_pool(name="adjp", bufs=adj_bufs) as ap_pool,
            tc.tile_pool(name="small", bufs=2) as smp,
            tc.tile_pool(name="pzw", bufs=2 if pair_drain else 3,
                         space="PSUM") as pzw,
            tc.tile_pool(name="pdif", bufs=1, space="PSUM") as pdif,
            tc.tile_pool(name="ph", bufs=1, space="PSUM") as ph,
            tc.tile_pool(name="pz", bufs=1 if pair_drain else 2,
                         space="PSUM") as pz,
        ):
            # ---- persistent SBUF ----
            zt = pp.tile([D, N], BF16)
            zts = pp.tile([D, N], BF16)
            wpack = pp.tile([128, W_COLS], BF16)
            fpack = pp.tile([128, F_COLS], FP32)
            onesc = pp.tile([1, 1], FP32)
            ywct = pp.tile([128, JT * YW], BF16)
            t_all = pp.tile([128, JT], FP32)
            t_core = pp.tile([128, IT], FP32)
            psi_f = pp.tile([128, JT * PP], FP32)
            psi_b = pp.tile([128, JT * PP], BF16)
            psi_core = pp.tile([128, IT * PP], FP32)
            reaT = pp.tile([D, M], FP32)
            ht = pp.tile([D, M], BF16)
            pht_all = pp.tile([PP, IT * 128], BF16)
            out_sb = pp.tile([D, M], FP32)
            ones16 = pp.tile([PP, 128], BF16)
            ident = pp.tile([128, 128], FP32)

            rhsw = wpack[:, W_RHSW : W_RHSW + 258]
            w1t = wpack[:, W_W1T : W_W1T + 128]
            w2t = wpack[:, W_W2T : W_W2T + 128]
            mt = wpack[0:PP, W_MT : W_MT + PP]
            mtsh = wpack[0:PP, W_MTSH : W_MTSH + PP]
            zi = wpack[:, W_ZI : W_ZI + M]
            b1 = fpack[:, F_B1 : F_B1 + 1]
            m0row = fpack[0:1, F_M0 : F_M0 + PP]
            b2c = fpack[:, F_B2C : F_B2C + 1]
            discb = fpack[:, F_DISCB : F_DISCB + M]

            nc.scalar.dma_start(wpack[:], wpack_d[:])
            nc.scalar.dma_start(fpack[:], fpack_d[:])
            if not zt_in_loop:
                nc.sync.dma_start(zt[:], zt_d[:])
                nc.sync.dma_start(zts[:], zts_d[:])

            make_identity(nc, ident[:])
            nc.vector.memset(onesc[:], 1.0)
            nc.vector.memset(ones16[:], 1.0)
            # power-basis pads: col 0 = 1.0, cols DEG+1..15 = 0
            nc.vector.memset(psi_f[:], 0.0)
            psi_v = psi_f[:].rearrange("p (t c) -> p t c", c=PP)
            nc.vector.memset(psi_v[:, :, 0], 1.0)
            nc.vector.memset(psi_core[:], 0.0)
            psc_v = psi_core[:].rearrange("p (t c) -> p t c", c=PP)
            nc.vector.memset(psc_v[:, :, 0], 1.0)

            ywct_v = ywct[:].rearrange("p (t c) -> p t c", c=YW)
            adj_fix = None
            if no_adj:
                adj_fix = pp.tile([128, slab * M], FP8)
                nc.vector.memset(adj_fix[:], 1.0)

            # ---- PSUM accumulators (allocated once, cleared by start=True) ----
            ps_dif = [
                pdif.tile([128, 512], FP32, tag=f"dif{h}", name=f"ps_dif{h}")
                for h in range(2)
            ]
            ps_h = ph.tile([PP, 129], FP32)

            import contextlib
            rep_ctx = tc.For_i(0, reps, 1) if hwloop and reps > 1 else None
            for _rep in range(1 if hwloop else reps):
              with (rep_ctx if rep_ctx is not None else contextlib.nullcontext()):
                # ---- pre-loop: own-row t basis + reaction MLP (zi only) ----
                for it in range(IT):
                    tps = pz.tile([128, 2], FP32, tag="mix", name="tps")
                    nc.tensor.matmul(
                        tps[:], zi[:, it * 128 : (it + 1) * 128], rhsw[:, 256:258],
                        start=True, stop=True, skip_group_check=True,
                    )
                    nc.vector.tensor_copy(t_core[:, it : it + 1], tps[:, 0:1])
                nc.vector.tensor_copy(psc_v[:, :, 1], t_core[:])
                for l in range(2, DEG + 1):
                    nc.vector.tensor_mul(
                        psc_v[:, :, l], psc_v[:, :, l - 1], t_core[:]
                    )
                for it in range(IT):
                    phps = pz.tile([PP, 128], FP32, tag="mix", name="phps")
                    nc.tensor.transpose(
                        phps[:], psi_core[:, it * PP : (it + 1) * PP], ident[:]
                    )
                    nc.vector.tensor_copy(
                        pht_all[:, it * 128 : (it + 1) * 128], phps[:]
                    )

                for hh in range(2):
                    t1 = pz.tile([128, 512], FP32, tag="mix", name="t1")
                    nc.tensor.matmul(
                        t1[:], w1t[:], zi[:, hh * 512 : (hh + 1) * 512],
                        start=True, stop=True, skip_group_check=True,
                    )
                    nc.scalar.activation(
                        ht[:, hh * 512 : (hh + 1) * 512], t1[:], AF.Tanh,
                        bias=b1[:], scale=1.0,
                    )
                for hh in range(2):
                    # rea^T = W2' @ tanh(...), b2 folded into the drain
                    rea_ps = pz.tile([128, 512], FP32, tag="mix", name="rea_ps")
                    nc.tensor.matmul(
                        rea_ps[:], w2t[:], ht[:, hh * 512 : (hh + 1) * 512],
                        start=True, stop=True, skip_group_check=True,
                    )
                    nc.vector.tensor_scalar(
                        reaT[:, hh * 512 : (hh + 1) * 512], rea_ps[:],
                        b2c[:], None, op0=ALU.add,
                    )

                # ---- j-tile stream: zw matmuls, merged drain, dif + moments ----
                # dif matmuls run at lag-1 in program order so the PE never
                # waits on its own tile's PSUM drain
                def dif_mms(jt, adjs, q, no_dif=no_dif):
                    if no_dif and jt > 0:
                        return
                    for h in range(2):
                        nc.tensor.matmul(
                            ps_dif[h][:],
                            ywct[:, jt * YW : jt * YW + 128],
                            adjs[:, q * M + h * 512 : q * M + (h + 1) * 512],
                            start=(jt == 0),
                            stop=(jt == JT - 1 or no_dif),
                            skip_group_check=True,
                        )

                prev = None
                for g in range(nslab):
                    # zt/zts chunks interleaved with adj slabs, 2 slabs per chunk
                    if zt_in_loop and g % 2 == 0:
                        csl = slice(g * slab * 128, (g + 2) * slab * 128)
                        nc.sync.dma_start(zt[:, csl], zt_d[:, csl])
                        nc.sync.dma_start(zts[:, csl], zts_d[:, csl])
                    if no_adj:
                        adjs = adj_fix
                    else:
                        adjs = ap_pool.tile([128, slab * M], FP8, tag="adj")
                        nc.sync.dma_start(
                            adjs[:], adj_d[:, g * slab * M : (g + 1) * slab * M]
                        )
                    if pair_drain:
                      for q in range(0, slab, 2):
                        # j-tile pair in one 2-bank PSUM tile, drained by a
                        # single strided copy (alternating DVE/ACT per pair)
                        zw2 = pzw.tile([128, 1024], FP32, tag="zw")
                        for u in range(2):
                            jt = g * slab + q + u
                            off = u * 512
                            nc.tensor.matmul(
                                zw2[:, off : off + 128],
                                zts[:, jt * 128 : (jt + 1) * 128],
                                rhsw[:, 0:128],
                                start=True, stop=False, skip_group_check=True,
                            )
                            nc.tensor.matmul(
                                zw2[:, off + 128 : off + 257],
                                zt[:, jt * 128 : (jt + 1) * 128],
                                rhsw[:, 128:257],
                                start=False, stop=True, skip_group_check=True,
                            )
                        jt0 = g * slab + q
                        zw2_v = zw2[:].rearrange("p (u c) -> p u c", c=512)
                        dst = ywct[:, jt0 * YW : (jt0 + 2) * YW].rearrange(
                            "p (u c) -> p u c", c=YW
                        )
                        if (jt0 // 2) % 2 == 0:
                            nc.vector.tensor_copy(dst, zw2_v[:, :, 0:YW])
                        else:
                            nc.scalar.copy(dst, zw2_v[:, :, 0:YW])
                        for u in range(2):
                            jt = jt0 + u
                            if prev is not None:
                                dif_mms(*prev)
                            prev = (jt, adjs, q + u)
                    else:
                      for q in range(slab):
                        jt = g * slab + q
                        zw = pzw.tile([128, YW], FP32, tag="zw")
                        nc.tensor.matmul(
                            zw[:, 0:128], zts[:, jt * 128 : (jt + 1) * 128],
                            rhsw[:, 0:128],
                            start=True, stop=False, skip_group_check=True,
                        )
                        nc.tensor.matmul(
                            zw[:, 128:257], zt[:, jt * 128 : (jt + 1) * 128],
                            rhsw[:, 128:257],
                            start=False, stop=True, skip_group_check=True,
                        )
                        use_dve = (
                            jt % 2 == 0 if drain_mode == "alt"
                            else drain_mode == "dve"
                        )
                        if use_dve:
                            nc.vector.tensor_copy(
                                ywct[:, jt * YW : (jt + 1) * YW], zw[:]
                            )
                        else:
                            nc.scalar.copy(
                                ywct[:, jt * YW : (jt + 1) * YW], zw[:]
                            )
                        if prev is not None:
                            dif_mms(*prev)
                        prev = (jt, adjs, q)
                    # psi powers (DVE, batched) + moment matmuls, every
                    # NSLAB//psi_batches slabs so H stays incremental
                    per = nslab // psi_batches
                    if (g + 1) % per == 0:
                        bsl = slice((g + 1 - per) * slab, (g + 1) * slab)
                        nc.vector.tensor_copy(t_all[:, bsl], ywct_v[:, bsl, 256])
                        nc.vector.tensor_copy(psi_v[:, bsl, 1], t_all[:, bsl])
                        for l in range(2, DEG + 1):
                            nc.vector.tensor_mul(
                                psi_v[:, bsl, l], psi_v[:, bsl, l - 1],
                                t_all[:, bsl],
                            )
                        nc.vector.tensor_copy(
                            psi_b[:, bsl.start * PP : bsl.stop * PP],
                            psi_f[:, bsl.start * PP : bsl.stop * PP],
                        )
                        for jt in range(bsl.start, bsl.stop):
                            nc.tensor.matmul(
                                ps_h[:],
                                psi_b[:, jt * PP : (jt + 1) * PP],
                                ywct[:, jt * YW + 128 : (jt + 1) * YW],
                                start=(jt == 0), stop=(jt == JT - 1),
                                skip_group_check=True,
                            )

                if prev is not None:
                    dif_mms(*prev)

                # ---- G = M @ H (+ denominator from shifted moments) ----
                hsb = smp.tile([PP, 129], BF16, tag="hsb")
                nc.vector.tensor_copy(hsb[:], ps_h[:])
                gp = pz.tile([PP, 129], FP32, tag="mix", name="gp")
                nc.tensor.matmul(gp[:, 0:128], mt[:], hsb[:, 0:128],
                                 start=True, stop=False, skip_group_check=True)
                nc.tensor.matmul(gp[:, 128:129], mtsh[:], hsb[:, 128:129],
                                 start=False, stop=False, skip_group_check=True)
                nc.tensor.matmul(gp[:, 128:129], m0row[:], onesc[:],
                                 start=False, stop=True, skip_group_check=True)
                gsb = smp.tile([PP, 129], BF16, tag="gsb")
                nc.vector.tensor_copy(gsb[:], gp[:])

                # ---- finish (all in [D, M] orientation, no transposes) ----
                # con^T numerator, denominator broadcast, reciprocal: all
                # independent of the dif stream, so they overlap its tail
                gbc = smp.tile([PP, 128], BF16, tag="gbc")
                nc.vector.tensor_scalar(
                    gbc[:], ones16[:], gp[:, 128:129], None, op0=ALU.mult
                )
                cps, rps = [], []
                cpool = pzw if pair_drain else pz
                ctag = "zw" if pair_drain else "mix"
                for h in range(2):
                    conp = cpool.tile([128, 512], FP32, tag=ctag, name="conp")
                    nc.tensor.matmul(
                        conp[:], gsb[:, 0:128],
                        pht_all[:, h * 512 : (h + 1) * 512],
                        start=True, stop=True, skip_group_check=True,
                    )
                    denp = cpool.tile([128, 512], FP32, tag=ctag, name="denp")
                    nc.tensor.matmul(
                        denp[:], gbc[:], pht_all[:, h * 512 : (h + 1) * 512],
                        start=True, stop=True, skip_group_check=True,
                    )
                    rcph = smp.tile([128, 512], FP32, tag="rcp")
                    nc.vector.reciprocal(rcph[:], denp[:])
                    cps.append(conp)
                    rps.append(rcph)
                # dif-dependent tail: relu(dif^T) . dinv_i + con + rea
                for h in range(2):
                    sl = slice(h * 512, (h + 1) * 512)
                    dd = smp.tile([128, 512], FP32, tag="dd")
                    nc.vector.tensor_tensor(
                        dd[:], ps_dif[h][:], discb[:, sl], op=ALU.mult
                    )
                    o1 = smp.tile([128, 512], FP32, tag="o1")
                    nc.vector.tensor_scalar(o1[:], dd[:], 0.0, None, op0=ALU.max)
                    cc = smp.tile([128, 512], FP32, tag="cc")
                    nc.vector.tensor_tensor(cc[:], cps[h][:], rps[h][:], op=ALU.mult)
                    o2 = smp.tile([128, 512], FP32, tag="o2")
                    nc.vector.tensor_add(o2[:], o1[:], cc[:])
                    nc.vector.tensor_add(out_sb[:, sl], o2[:], reaT[:, sl])
                nc.sync.dma_start(out_d[:], out_sb[:])

    nc.compile()
    return nc


def _sigmoid(x):
    return 1.0 / (1.0 + np.exp(-np.float64(x)))


def prep_inputs(Z, A_norm, W_D, W_C, w_V, W1, b1, W2, b2, omega_logit, delta_logit):
    import ml_dtypes

    bf16 = ml_dtypes.bfloat16
    f8 = mybir.dt.np(FP8)

    Z = np.asarray(Z, dtype=np.float32)
    A_norm = np.asarray(A_norm, dtype=np.float32)
    omega = _sigmoid(omega_logit)
    delta = _sigmoid(delta_logit)

    # attention polynomial: fit exp(sigmoid(x)) on x = s_i - s_j via
    # Chebyshev in u = (t_i - t_j)/2, t = s/S, then binomial expansion
    # into the separable coefficient matrix Mm[l, p].
    s = Z.astype(np.float64) @ np.asarray(w_V, np.float64)
    S = float(np.max(np.abs(s))) * 1.01
    cheb = np.polynomial.chebyshev.Chebyshev.interpolate(
        lambda u: np.exp(_sigmoid(2.0 * S * u)), DEG, domain=[-1, 1]
    )
    p = cheb.convert(kind=np.polynomial.Polynomial).coef
    Mm = np.zeros((PP, PP))
    for k in range(DEG + 1):
        for l in range(k + 1):
            Mm[l, k - l] += p[k] * (0.5 ** k) * comb(k, l) * ((-1.0) ** (k - l))

    # binary adjacency + degree factors
    mask = A_norm != 0
    degc = mask.sum(axis=1)
    dinv = np.where(degc > 0, 1.0 / np.sqrt(np.maximum(degc, 1)), 0.0).astype(
        np.float32
    )
    adj8 = mask.astype(f8)

    zt = np.ascontiguousarray(Z.T).astype(bf16)                    # [D, N]
    zts = np.ascontiguousarray(Z.T * dinv[None, :]).astype(bf16)   # dinv-scaled

    # packed bf16 small inputs (shared part; zi appended per core)
    wpack = np.zeros((128, W_COLS), dtype=bf16)
    wpack[:, W_RHSW : W_RHSW + 128] = (omega * np.asarray(W_D, np.float64)).T
    wpack[:, W_RHSW + 128 : W_RHSW + 256] = (
        (1.0 - omega) * np.asarray(W_C, np.float64)
    ).T
    wpack[:, W_RHSW + 256] = np.asarray(w_V, np.float64) / S
    wpack[:, W_W1T : W_W1T + 128] = np.asarray(W1, np.float64).T
    wpack[:, W_W2T : W_W2T + 128] = (delta * np.asarray(W2, np.float64)).T
    wpack[0:PP, W_MT : W_MT + PP] = Mm.T
    Msh = np.zeros((PP, PP))
    Msh[0 : PP - 1, :] = Mm.T[1:PP, :]
    wpack[0:PP, W_MTSH : W_MTSH + PP] = Msh

    fpack = np.zeros((128, F_COLS), dtype=np.float32)
    fpack[:, F_B1] = np.asarray(b1, np.float32)
    fpack[0, F_M0 : F_M0 + PP] = (Mm[:, 0] * float(N)).astype(np.float32)
    fpack[:, F_B2C] = (delta * np.asarray(b2, np.float64)).astype(np.float32)

    shared = {"zt": zt, "zts": zts}
    in_maps = []
    for c in range(NCORES):
        sl = slice(c * M, (c + 1) * M)
        # column slice, swizzled to j-tile layout [128, JT*M]
        adj_c = adj8[:, sl].reshape(JT, 128, M).transpose(1, 0, 2).reshape(
            128, JT * M
        )
        wp = wpack.copy()
        wp[:, W_ZI : W_ZI + M] = zt[:, sl]
        fp = fpack.copy()
        fp[:, F_DISCB : F_DISCB + M] = dinv[sl][None, :]
        in_maps.append({
            **shared,
            "adj": np.ascontiguousarray(adj_c),
            "wpack": np.ascontiguousarray(wp),
            "fpack": np.ascontiguousarray(fp),
        })
    return in_maps


def kernel(Z, A_norm, W_D, W_C, w_V, W1, b1, W2, b2, omega_logit, delta_logit):
    global LAST_RESULTS, LAST_IN_MAPS
    in_maps = prep_inputs(
        Z, A_norm, W_D, W_C, w_V, W1, b1, W2, b2, omega_logit, delta_logit
    )
    LAST_IN_MAPS = in_maps
    nc = build_program()
    LAST_RESULTS = run_bass_kernel_spmd(nc, in_maps, list(range(NCORES)))
    # device emits out^T [D, M] per core
    return np.concatenate(
        [LAST_RESULTS.results[c]["out"].T for c in range(NCORES)], axis=0
    )
